# revision 1
# baseline (speedup 1.0000x reference)
"""Deformable-conv Trainium2 kernel v2 (nn_DeformConv_11553462026367).

Data-parallel over batch: one sample per NeuronCore (8 cores).

v2 design (vs v1): bf16 pipeline + transpose-mode dma_gather.
  - Gather table = pre-materialized 2x2 bilinear patch records, 256 bf16
    (512 B) each, record (c, r) at DRAM row c*132 + r holding
    [x[r-2,c-2], x[r-2,c-1], x[r-1,c-2], x[r-1,c-1]] channel-interleaved
    as elem = ch*4 + q.  Border records are zero, so OOB masking is free.
    Table is built ON DEVICE from x (PE transposes + strided DMAs).
  - dma_gather(transpose=True) returns [p=elem%128, m=elem//128, idx]:
    partitions = (ch%32)*4 + q, free = (ch//32, idx) -- channel-major, so
    NO per-position transposes are needed before the output matmul.
  - Bilinear blend = ONE bf16 tensor_tensor per (k, m-half): the quadrant
    weight tile W[p, i] = w_{p%4}[i] is expanded on the PE (one-hot SEL4
    matmul against transposed phase-B weights) and cast PSUM->SBUF bf16.
  - Output conv: accumulating bf16 matmuls, contraction over (ch%32, q),
    summed over k and both m-halves straight in PSUM.

kernel(**inputs) takes the FULL batch and returns the FULL output.
"""
import sys
sys.path.insert(0, "/opt/trn_rl_repo")

import numpy as np
import ml_dtypes
from contextlib import ExitStack

from concourse import bass, tile
import concourse.bacc as bacc
from concourse.tile import add_dep_helper
import concourse.bass_utils as bass_utils
import concourse.mybir as mybir
from concourse.masks import make_identity

F32 = mybir.dt.float32
F32R = mybir.dt.float32r
BF16 = mybir.dt.bfloat16
I32 = mybir.dt.int32
I16 = mybir.dt.int16
ALU = mybir.AluOpType

# ---- problem constants (hardcoded; kernel.py must be self-contained) ----
B, C, H, W = 8, 64, 128, 128
KK = 9
HW = H * W                 # 16384 positions
LRr = 132                  # records per table column: r = clamp(y0+2) in [0,131]
LCc = 133                  # table columns: c = clamp(x0+2) in [0,132]
NREC = LCc * LRr           # 17556 records
NRECP = NREC + 8           # pad
CAST_RNE = True            # HW f32->i32 tensor_copy rounds-to-nearest (sim truncates)
NCORES = 8

NI = 2048                  # gather indices per call
NCHUNK = HW // NI          # 8
SS = NI // 128             # 16 s-slots (of 128 positions) per chunk


def _mm(ap, dt=F32R):
    return ap.bitcast(dt) if dt != F32 else ap


def build_kernel(tc, outs, ins):
    nc = tc.nc
    ctx = ExitStack()
    with ctx:
        # ---------------- constants ----------------
        const_pool = ctx.enter_context(tc.tile_pool(name="const", bufs=1))
        ident = const_pool.tile([128, 128], F32)
        make_identity(nc, ident[:])
        identb = const_pool.tile([128, 128], BF16)
        nc.scalar.copy(identb[:], ident[:])

        piota_i = const_pool.tile([128, 1], I32)
        nc.gpsimd.iota(piota_i[:], pattern=[[0, 1]], base=0, channel_multiplier=1)
        piota = const_pool.tile([128, 1], F32)
        nc.vector.tensor_copy(piota[:], piota_i[:])
        siota_i = const_pool.tile([128, 128], I32)
        nc.gpsimd.iota(siota_i[:], pattern=[[1, 128]], base=0, channel_multiplier=0)
        siota = const_pool.tile([128, 128], F32)
        nc.vector.tensor_copy(siota[:], siota_i[:])

        # msel[pp][q, P] = 1 iff q == pp*16 + P%16  (wrap-permute one-hots)
        msel_f = const_pool.tile([128, 8 * 128], F32)
        msel_v = msel_f[:].rearrange("p (a b) -> p a b", a=8)
        clo16_i = const_pool.tile([128, 128], I32)
        nc.vector.tensor_scalar(clo16_i[:], siota_i[:], 15, None, ALU.bitwise_and)
        clo16 = const_pool.tile([128, 128], F32)
        nc.vector.tensor_copy(clo16[:], clo16_i[:])
        psh = const_pool.tile([128, 8], F32)
        for pp in range(8):
            nc.vector.tensor_scalar(psh[:, pp:pp + 1], piota[:], float(pp * 16),
                                    None, ALU.subtract)
            nc.vector.tensor_scalar(msel_v[:, pp, :], clo16[:], psh[:, pp:pp + 1],
                                    None, ALU.is_equal)

        # SEL36_k[(q*9+j), p] = 1 iff j == k and p%4 == q: selects k's four
        # quadrant rows out of the [36, *] chunk-weight tile.
        sel36_f = const_pool.tile([36, 9 * 128], F32)
        s36v = sel36_f[:].rearrange("p (k c) -> p k c", k=KK)
        clo4_36i = const_pool.tile([36, 128], I32)
        nc.vector.tensor_scalar(clo4_36i[:], siota_i[:36, :], 3, None,
                                ALU.bitwise_and)
        clo4_36 = const_pool.tile([36, 128], F32)
        nc.vector.tensor_copy(clo4_36[:], clo4_36i[:])
        qk_f = const_pool.tile([36, 1], F32)
        nc.vector.tensor_copy(qk_f[:], piota_i[:36, :])
        for k in range(KK):
            # partition q*9+j selects columns p with j == k, p%4 == q
            t36 = const_pool.tile([36, 128], F32)
            nc.vector.tensor_scalar(t36[:], clo4_36[:], 9.0, float(k),
                                    ALU.mult, ALU.add)
            nc.vector.tensor_scalar(s36v[:, k, :], t36[:], qk_f[:], None,
                                    ALU.is_equal)
        sel36 = const_pool.tile([36, 9 * 128], BF16)
        nc.scalar.copy(sel36[:], sel36_f[:])
        sel36_v = sel36[:].rearrange("p (k c) -> p k c", k=KK)

        # weights from host (offset-conv taps paired two-per-matmul)
        woff_f = const_pool.tile([128, 6 * 18], F32)
        nc.sync.dma_start(
            woff_f[:].rearrange("p (k o) -> p k o", k=6),
            ins["wpair"].transpose([1, 0, 2]))
        woff_sb = const_pool.tile([128, 6 * 18], F32R)
        nc.scalar.copy(woff_sb[:], woff_f[:])

        wdrep = const_pool.tile([128, 18 * 64], BF16)
        nc.sync.dma_start(
            wdrep[:].rearrange("p (i o) -> p i o", i=18),
            ins["wdrep"].transpose([1, 0, 2]))
        wdrep_v = wdrep[:].rearrange("p (i o) -> p i o", i=18)

        # ---------------- persistent tiles ----------------
        T_pool = ctx.enter_context(tc.tile_pool(name="persist", bufs=1))
        Ttile = T_pool.tile([128, 128 * 18], F32)          # offsets [p=w, s=h, ch]
        T3 = Ttile[:].rearrange("p (s c) -> p s c", c=18)
        W16all = T_pool.tile([128, KK * NCHUNK * 128], I16)  # wrapped gather idx
        W16v = W16all[:].rearrange("p (k c t) -> p k c t", k=KK, c=NCHUNK)
        Tsb = T_pool.tile([128, KK * 4 * 128], BF16)       # transposed quad weights
        # per-k full-s index-math results, filled per conv quarter
        fl9 = T_pool.tile([128, KK * 128], F32)
        fy9 = T_pool.tile([128, KK * 128], BF16)
        fx9 = T_pool.tile([128, KK * 128], BF16)
        fl9v = fl9[:].rearrange("p (k s) -> p k s", k=KK)
        fy9v = fy9[:].rearrange("p (k s) -> p k s", k=KK)
        fx9v = fx9[:].rearrange("p (k s) -> p k s", k=KK)

        tab = ins["tab"]  # internal DRAM [NRECP, 256] bf16

        # ================= prologue: xpad + offsets conv + table =========
        with tc.tile_pool(name="xpad", bufs=1) as xpad_pool:
            xpad_t = xpad_pool.tile([64, 130 * 130], F32)
            xpv = xpad_t[:].rearrange("p (r c) -> p r c", c=130)
            nc.vector.memset(xpv[:, 0, :], 0.0)
            nc.vector.memset(xpv[:, 129, :], 0.0)
            nc.vector.memset(xpv[:, 1:129, 0], 0.0)
            nc.vector.memset(xpv[:, 1:129, 129], 0.0)
            xin = ins["x"].rearrange("p (h w) -> p h w", w=128)
            for qh in range(8):
                nc.sync.dma_start(
                    xpv[:, 1 + qh * 16:1 + (qh + 1) * 16, 1:129],
                    xin[:, qh * 16:(qh + 1) * 16, :])

            # ---- offsets conv + transposed-x table build, interleaved ----
            tab_dmas = []
            with tc.tile_pool(name="offsb", bufs=1) as offsb_pool, \
                 tc.tile_pool(name="xr", bufs=2) as xr_pool, \
                 tc.tile_pool(name="offps", bufs=3, space="PSUM") as offps_pool, \
                 tc.tile_pool(name="trps", bufs=2, space="PSUM") as trps_pool, \
                 tc.tile_pool(name="xta", bufs=1) as xta_pool, \
                 tc.tile_pool(name="xt4", bufs=1) as xt4_pool, \
                 tc.tile_pool(name="ixq", bufs=2) as ixp2, \
                 tc.tile_pool(name="tps", bufs=2, space="PSUM") as tps_pool:
                offs_sb = offsb_pool.tile([18, HW // 4], F32)
                woff_v = woff_sb[:].rearrange("p (k o) -> p k o", k=6)
                # tap pairs (0,1) (3,4) (6,7) differ by +1 in kx, so one
                # +1-shifted second rhs half serves all three 128-deep
                # matmuls; k2/k5/k8 stay 64-deep singles
                PAIRS = [(0, True), (3, True), (6, True),
                         (2, False), (5, False), (8, False)]
                # XTa0[p=col, (y, ch)] = x[y, p];  XTa1[p] = x[y, p+1]
                # (col shift comes from the transpose source offset; xpad's
                #  zero pad column supplies x[y, 128] = 0 automatically)
                XTa0 = xta_pool.tile([128, 128 * 64], BF16)
                XTa1 = xta_pool.tile([128, 128 * 64], BF16)
                XTav0 = XTa0[:].rearrange("p (y c) -> p y c", c=64)
                XTav1 = XTa1[:].rearrange("p (y c) -> p y c", c=64)

                def emit_xta(quarter):
                    for sh, XT in ((1, XTa0), (2, XTa1)):
                        for yb in range(quarter * 4, quarter * 4 + 4):
                            tp = tps_pool.tile([128, 512], F32, tag="tps")
                            for t in range(8):
                                y = yb * 8 + t
                                src = bass.AP(xpad_t.tensor,
                                              xpad_t[:].offset + (y + 1) * 130 + sh,
                                              [[130 * 130, 64], [1, 128]])
                                nc.tensor.transpose(tp[:, t * 64:(t + 1) * 64],
                                                    src, ident[:64, :64])
                            if yb % 2 == 0:
                                nc.vector.tensor_copy(
                                    XT[:, yb * 512:(yb + 1) * 512], tp[:])
                            else:
                                nc.scalar.copy(
                                    XT[:, yb * 512:(yb + 1) * 512], tp[:])

                def emit_xt4(qq):
                    for hh in range(2):
                        XT4 = xt4_pool.tile([128, 16 * 256], BF16, tag="xt4")
                        x4q = XT4[:].rearrange("p (y c q) -> p y c q", c=64, q=4)
                        y0 = qq * 32 + hh * 16
                        ylim = 16 if y0 + 16 < 128 else 15
                        # q0: x[y,p]  q1: x[y,p+1]  q2: x[y+1,p]  q3: x[y+1,p+1]
                        nc.gpsimd.tensor_copy(x4q[:, :, :, 0], XTav0[:, y0:y0 + 16, :])
                        nc.gpsimd.tensor_copy(x4q[:, :, :, 1], XTav1[:, y0:y0 + 16, :])
                        nc.gpsimd.tensor_copy(x4q[:, :ylim, :, 2],
                                              XTav0[:, y0 + 1:y0 + 1 + ylim, :])
                        nc.gpsimd.tensor_copy(x4q[:, :ylim, :, 3],
                                              XTav1[:, y0 + 1:y0 + 1 + ylim, :])
                        if y0 + 16 == 128:
                            nc.vector.memset(x4q[:, 15, :, 2], 0.0)
                            nc.vector.memset(x4q[:, 15, :, 3], 0.0)
                        # records (c=p+2, r=y0+2+y)
                        d = nc.sync.dma_start(
                            bass.AP(tab.tensor, (2 * LRr + y0 + 2) * 256,
                                    [[LRr * 256, 128], [1, 16 * 256]]),
                            XT4[:])
                        tab_dmas.append(d)

                def emit_idx(qq):
                    # index math for this quarter's 32 s-columns, all k
                    sl = slice(qq * 32, qq * 32 + 32)
                    rne = 0.5 if CAST_RNE else 0.0
                    for k in range(KK):
                        ky, kx = k // 3, k % 3
                        dy = T3[:, sl, 2 * k]
                        dx = T3[:, sl, 2 * k + 1]
                        ysp8 = ixp2.tile([128, 32], F32, tag="ysp8")
                        nc.vector.tensor_tensor(ysp8[:], dy, siota[:, sl], ALU.add)
                        nc.vector.tensor_scalar(ysp8[:], ysp8[:],
                                                float(ky + 7) - rne, None, ALU.add)
                        yint = ixp2.tile([128, 32], I32, tag="yint")
                        nc.vector.tensor_copy(yint[:], ysp8[:])
                        y0f = ixp2.tile([128, 32], F32, tag="y0f")
                        nc.vector.tensor_copy(y0f[:], yint[:])
                        # fold the RNE +0.5 restore into the subtract
                        nc.vector.scalar_tensor_tensor(
                            fy9v[:, k, sl], ysp8[:], 0.5 if CAST_RNE else 0.0,
                            y0f[:], ALU.add, ALU.subtract)
                        yi = ixp2.tile([128, 32], F32, tag="yi")
                        nc.vector.tensor_scalar(yi[:], y0f[:], 6.0, 137.0,
                                                ALU.max, ALU.min)

                        xsp8 = ixp2.tile([128, 32], F32, tag="xsp8")
                        nc.vector.tensor_scalar(xsp8[:], dx, piota[:],
                                                float(kx + 7) - rne,
                                                ALU.add, ALU.add)
                        xint = ixp2.tile([128, 32], I32, tag="xint")
                        nc.vector.tensor_copy(xint[:], xsp8[:])
                        x0f = ixp2.tile([128, 32], F32, tag="x0f")
                        nc.vector.tensor_copy(x0f[:], xint[:])
                        nc.vector.scalar_tensor_tensor(
                            fx9v[:, k, sl], xsp8[:], 0.5 if CAST_RNE else 0.0,
                            x0f[:], ALU.add, ALU.subtract)
                        xi = ixp2.tile([128, 32], F32, tag="xi")
                        nc.vector.tensor_scalar(xi[:], x0f[:], 6.0, 138.0,
                                                ALU.max, ALU.min)
                        # record idx = (xi-6)*132 + (yi-6)
                        nc.vector.tensor_scalar(fl9v[:, k, sl], xi[:], 132.0,
                                                float(6 * 132 + 6),
                                                ALU.mult, ALU.subtract)
                        nc.vector.tensor_tensor(fl9v[:, k, sl], fl9v[:, k, sl],
                                                yi[:], ALU.add)

                zband = {}

                def emit_zero_bands():
                    dza = nc.sync.dma_start(
                        bass.AP(tab.tensor, 0, [[1, 264 * 256]]),
                        bass.AP(zt.tensor, zt[:].offset, [[808, 128], [1, 528]]))
                    tab_dmas.append(dza)
                    d = nc.sync.dma_start(
                        bass.AP(tab.tensor, 17160 * 256,
                                [[1, (NRECP - 17160) * 256]]),
                        bass.AP(zt.tensor, zt[:].offset, [[808, 128], [1, 808]]))
                    tab_dmas.append(d)
                    ztc = xta_pool.tile([128, 512], BF16)
                    nc.vector.memset(ztc[:], 0.0)
                    dz1 = nc.sync.dma_start(
                        bass.AP(tab.tensor, LRr * 256,
                                [[LRr * 256, 128], [1, 512]]),
                        ztc[:])
                    tab_dmas.append(dz1)
                    d = nc.sync.dma_start(
                        bass.AP(tab.tensor, 129 * LRr * 256, [[1, 512]]),
                        ztc[0:1, :])
                    tab_dmas.append(d)
                    d = nc.sync.dma_start(
                        bass.AP(tab.tensor, (LRr + 130) * 256,
                                [[LRr * 256, 128], [1, 512]]),
                        ztc[:])
                    tab_dmas.append(d)
                    d = nc.sync.dma_start(
                        bass.AP(tab.tensor, (129 * LRr + 130) * 256, [[1, 512]]),
                        ztc[0:1, :])
                    tab_dmas.append(d)
                    zband["dza"] = dza
                    zband["dz1"] = dz1

                # border-record zero bands fire first (independent of x)
                zt = xta_pool.tile([128, 808], BF16)
                nc.vector.memset(zt[:], 0.0)
                emit_zero_bands()

                for quarter in range(4):
                    emit_xta(quarter)
                    if quarter >= 1:
                        emit_xt4(quarter - 1)
                    for hch in range(8):       # chunks of 4 image rows
                        cch = quarter * 8 + hch
                        y0 = cch * 4
                        xr = xr_pool.tile([128, 6 * 130], F32R, tag="xr")
                        nc.scalar.copy(xr[:64, :],
                                       xpad_t[:, y0 * 130:(y0 + 6) * 130])
                        ln = min(780, 130 * 130 - (y0 * 130 + 1))
                        nc.gpsimd.tensor_copy(
                            bass.AP(xr.tensor, xr[:].offset + 64 * 780,
                                    [[780, 64], [1, ln]]),
                            xpad_t[:, y0 * 130 + 1:y0 * 130 + 1 + ln])
                        ps = offps_pool.tile([18, 512], F32, tag="offps")
                        for i, (ka, paired) in enumerate(PAIRS):
                            ky, kx = ka // 3, ka % 3
                            nparts = 128 if paired else 64
                            src = bass.AP(
                                xr.tensor, xr[:].offset + ky * 130 + kx,
                                [[6 * 130, nparts], [130, 4], [1, 128]])
                            nc.tensor.matmul(
                                ps[:], woff_v[:nparts, i, :], src,
                                start=(i == 0), stop=(i == len(PAIRS) - 1))
                        nc.scalar.copy(offs_sb[:, hch * 512:(hch + 1) * 512], ps[:])

                    for sh4 in range(8):
                        tp = trps_pool.tile([128, 4 * 18], F32, tag="trps")
                        for j4 in range(4):
                            sh = sh4 * 4 + j4
                            nc.tensor.transpose(
                                tp[:, j4 * 18:(j4 + 1) * 18],
                                offs_sb[:, sh * 128:(sh + 1) * 128], ident[:18, :18])
                        s = quarter * 32 + sh4 * 4
                        nc.scalar.copy(
                            T3[:, s:s + 4, :],
                            tp[:].rearrange("p (a c) -> p a c", a=4))

                    emit_idx(quarter)
                emit_xt4(3)


                # c=1 column: q1 = x[y, 0], q3 = x[y+1, 0]  (partition = y)
                srec = xta_pool.tile([128, 256], BF16)
                sv = srec[:].rearrange("p (c q) -> p c q", q=4)
                tp1 = tps_pool.tile([128, 128], F32, tag="tps")
                src = bass.AP(xpad_t.tensor, xpad_t[:].offset + 131,
                              [[130 * 130, 64], [130, 128]])
                nc.tensor.transpose(tp1[:, 0:64], src, ident[:64, :64])
                src = bass.AP(xpad_t.tensor, xpad_t[:].offset + 261,
                              [[130 * 130, 64], [130, 128]])
                nc.tensor.transpose(tp1[:, 64:128], src, ident[:64, :64])
                nc.vector.memset(sv[:, :, 0], 0.0)
                nc.vector.memset(sv[:, :, 2], 0.0)
                nc.scalar.copy(sv[:, :, 1], tp1[:, 0:64])
                nc.scalar.copy(sv[:, :, 3], tp1[:, 64:128])
                dsr = nc.sync.dma_start(
                    bass.AP(tab.tensor, (LRr + 2) * 256, [[256, 128], [1, 256]]),
                    srec[:])
                add_dep_helper(dsr.ins, zband["dza"].ins, reason="c1 after zero band")
                tab_dmas.append(dsr)

                # r=1 row: q2 = x[0, p], q3 = x[0, p+1]
                srec2 = xta_pool.tile([128, 256], BF16)
                s2 = srec2[:].rearrange("p (c q) -> p c q", q=4)
                nc.vector.memset(s2[:, :, 0], 0.0)
                nc.vector.memset(s2[:, :, 1], 0.0)
                nc.scalar.copy(s2[:, :, 2], XTav0[:, 0, :])
                nc.scalar.copy(s2[:, :, 3], XTav1[:, 0, :])
                dsr2 = nc.sync.dma_start(
                    bass.AP(tab.tensor, (2 * LRr + 1) * 256,
                            [[LRr * 256, 128], [1, 256]]),
                    srec2[:])
                add_dep_helper(dsr2.ins, zband["dz1"].ins, reason="r1 after zero band")
                tab_dmas.append(dsr2)

                # corner record (c=1, r=1): q3 = x[0, 0]
                srec3 = xta_pool.tile([1, 256], BF16)
                s3 = srec3[:].rearrange("p (c q) -> p c q", q=4)
                nc.vector.memset(srec3[:], 0.0)
                nc.scalar.copy(s3[:, :, 3], XTav0[:1, 0, :])
                dsr3 = nc.sync.dma_start(
                    bass.AP(tab.tensor, (LRr + 1) * 256, [[1, 256]]),
                    srec3[:])
                add_dep_helper(dsr3.ins, zband["dza"].ins, reason="corner after zero")
                add_dep_helper(dsr3.ins, zband["dz1"].ins, reason="corner after r-band zero")
                tab_dmas.append(dsr3)

                # funnel: one touch op depending on every table DMA
                tdummy = xta_pool.tile([1, 2], BF16)
                touch_tab = nc.vector.memset(tdummy[:], 0.0)
                for d in tab_dmas:
                    add_dep_helper(touch_tab.ins, d.ins, reason="table barrier")

        # ================= phase B: index + weight math ==================
        # (includes per-k quad-weight transposes staged to rdram[q, k, s, p],
        #  so one DMA per chunk later reloads them as a [36, *] matmul rhs)
        rdram = ins["rdram"]
        r_dmas = []
        Wq4 = T_pool.tile([128, KK * 4 * 128], BF16)
        Wq4v = Wq4[:].rearrange("p (k q s) -> p k q s", k=KK, q=4)
        with tc.tile_pool(name="ixtmp", bufs=1) as ixp, \
             tc.tile_pool(name="wrps", bufs=2, space="PSUM") as wrp_pool, \
             tc.tile_pool(name="tq", bufs=2, space="PSUM") as tq_pool:
            # wrap record indices first: they gate the gather stream
            for k in range(KK):
                for pp in range(8):
                    wps = wrp_pool.tile([128, 128], F32, tag="wrps")
                    nc.tensor.matmul(wps[:], msel_v[:, pp, :], fl9v[:, k, :],
                                     start=True, stop=True)
                    dstw = bass.AP(W16all.tensor,
                                   W16all[:].offset + k * (NCHUNK * 128) + pp,
                                   [[KK * NCHUNK * 128, 128], [128, NCHUNK], [8, SS]])
                    if pp % 2 == 0:
                        nc.vector.tensor_copy(dstw, wps[:].rearrange(
                            "p (c u) -> p c u", c=NCHUNK))
                    else:
                        nc.scalar.copy(dstw, wps[:].rearrange(
                            "p (c u) -> p c u", c=NCHUNK))

            for k in range(KK):
                fy = fy9v[:, k, :]
                fx = fx9v[:, k, :]
                # quadrant weights (record order: q0=y0x0 q1=y0x1 q2=y1x0 q3=y1x1)
                wy0 = ixp.tile([128, 128], F32, tag="wy0")
                nc.vector.tensor_scalar(wy0[:], fy, -1.0, 1.0, ALU.mult, ALU.add)
                wx0 = ixp.tile([128, 128], F32, tag="wx0")
                nc.vector.tensor_scalar(wx0[:], fx, -1.0, 1.0, ALU.mult, ALU.add)
                nc.vector.tensor_tensor(Wq4v[:, k, 0, :], wy0[:], wx0[:], ALU.mult)
                nc.vector.tensor_tensor(Wq4v[:, k, 1, :], wy0[:], fx, ALU.mult)
                nc.vector.tensor_tensor(Wq4v[:, k, 2, :], fy, wx0[:], ALU.mult)
                nc.vector.tensor_tensor(Wq4v[:, k, 3, :], fy, fx, ALU.mult)

                # transpose quad weights -> Tsb[s, (k, q, p)] -> rdram
                tpq = tq_pool.tile([128, 512], BF16, tag="tq")
                for q in range(4):
                    nc.tensor.transpose(tpq[:, q * 128:(q + 1) * 128],
                                        Wq4v[:, k, q, :], identb[:])
                nc.scalar.copy(Tsb[:, k * 512:(k + 1) * 512], tpq[:])
                d = nc.sync.dma_start(
                    bass.AP(rdram.tensor, k * 16384,
                            [[128, 128], [KK * 16384, 4], [1, 128]]),
                    Tsb[:, k * 512:(k + 1) * 512])
                r_dmas.append(d)

        rdummy = T_pool.tile([1, 2], BF16)
        touch_r = nc.vector.memset(rdummy[:], 0.0)
        for d in r_dmas:
            add_dep_helper(touch_r.ins, d.ins, reason="rdram barrier")

        # ================= main loop: gather / blend / conv ==============
        with tc.tile_pool(name="g", bufs=10) as g_pool, \
             tc.tile_pool(name="wsb", bufs=2) as wsb_pool, \
             tc.tile_pool(name="rsl", bufs=2) as r_pool, \
             tc.tile_pool(name="osb", bufs=2) as osb_pool, \
             tc.tile_pool(name="tch", bufs=2) as tch_pool, \
             tc.tile_pool(name="wps2", bufs=1, space="PSUM") as wps_pool, \
             tc.tile_pool(name="ops", bufs=1, space="PSUM") as ops_pool:

            tab_src = bass.AP(tab.tensor, 0, [[256, NRECP], [1, 256]])
            ni_reg = nc.gpsimd.to_reg(NI)

            # last 2048-chunk runs as two 1024 halves to shorten the drain
            # after the final gather
            ni_reg_h = nc.gpsimd.to_reg(NI // 2)
            iters = [(ch, 0, NI) for ch in range(NCHUNK - 1)]
            iters += [(NCHUNK - 1, 0, NI // 2), (NCHUNK - 1, NI // 2, NI // 2)]
            for ch, off, n in iters:
                OPS = ops_pool.tile([64, n], F32, tag="ops")
                # this chunk's weight rows in one DMA: [36=(q,k), (s, p)]
                R36 = r_pool.tile([36, n], BF16, tag="rsl")
                dr = nc.sync.dma_start(
                    R36[:].rearrange("p (s c) -> p s c", c=128),
                    bass.AP(rdram.tensor, ch * NI + off,
                            [[16384, 4 * KK], [128, n // 128], [1, 128]]))
                add_dep_helper(dr.ins, touch_r.ins, reason="R after rdram")
                for k in range(KK):
                    G = g_pool.tile([128, 2 * n], BF16, tag="g")
                    G3 = G[:].rearrange("p (m i) -> p m i", m=2)
                    gi = nc.gpsimd.dma_gather(
                        G3, tab_src, W16v[:, k, ch, off // 16:(off + n) // 16],
                        n, ni_reg if n == NI else ni_reg_h, 256,
                        transpose=True, single_packet=False)
                    add_dep_helper(gi.ins, touch_tab.ins, reason="gather after table")
                    touch = tch_pool.tile([128, 2], BF16, tag="tch")
                    touch_i = nc.vector.tensor_copy(touch[:], G3[:, 0, 0:2])

                    # expand W[p, i] = R36[(p%4)*9+k, i] on PE
                    WPS = wps_pool.tile([128, n], F32, tag="wps2")
                    for g4 in range(n // 512):
                        nc.tensor.matmul(WPS[:, g4 * 512:(g4 + 1) * 512],
                                         sel36_v[:, k, :],
                                         R36[:, g4 * 512:(g4 + 1) * 512],
                                         start=True, stop=True)
                    Wsb = wsb_pool.tile([128, n], BF16, tag="wsb")
                    nc.scalar.copy(Wsb[:], WPS[:])

                    # blend (in place over G), then accumulate output conv
                    for m in range(2):
                        bl = nc.vector.tensor_tensor(G3[:, m, :], G3[:, m, :],
                                                     Wsb[:], ALU.mult)
                        if m == 0:
                            add_dep_helper(bl.ins, touch_i.ins, sync=False,
                                           reason="order blend after gather-touch")
                    for m in range(2):
                        for g4 in range(n // 512):
                            nc.tensor.matmul(
                                OPS[:, g4 * 512:(g4 + 1) * 512],
                                wdrep_v[:, k * 2 + m, :],
                                G3[:, m, g4 * 512:(g4 + 1) * 512],
                                start=(k == 0 and m == 0),
                                stop=(k == KK - 1 and m == 1))

                osb = osb_pool.tile([64, n], F32, tag="osb")
                nc.scalar.copy(osb[:], OPS[:])
                nc.sync.dma_start(
                    outs["out"][:, ch * NI + off:ch * NI + off + n], osb[:])


# ======================= host-side wrapper =======================

def prep_core_inputs(xb, w_offset, w_deform):
    """Per-core device inputs from one sample (cheap: no gather table)."""
    C_ = xb.shape[0]
    x = np.ascontiguousarray(xb.reshape(C_, -1), dtype=np.float32)

    woff = w_offset.reshape(18, C_, KK).transpose(2, 1, 0)   # [k, c, 18]
    wpair = np.zeros((6, 128, 18), np.float32)
    for i, (ka, kb) in enumerate([(0, 1), (3, 4), (6, 7),
                                  (2, None), (5, None), (8, None)]):
        wpair[i, 0:64, :] = woff[ka]
        if kb is not None:
            wpair[i, 64:128, :] = woff[kb]
    wpair = np.ascontiguousarray(wpair)

    wd = w_deform.reshape(64, C_, KK)          # [o, c, k]
    wdt = wd.transpose(2, 1, 0)                # [k, c, o]
    arr = np.zeros((KK, 2, 32, 4, 64), np.float32)
    arr[:] = wdt.reshape(KK, 2, 32, 1, 64)
    wdrep = np.ascontiguousarray(
        arr.reshape(KK * 2, 128, 64)).astype(ml_dtypes.bfloat16)
    return {"x": x, "wpair": wpair, "wdrep": wdrep}


_NC_CACHE = {}


def _build_nc():
    if "nc" in _NC_CACHE:
        return _NC_CACHE["nc"]
    nc = bacc.Bacc("TRN2", target_bir_lowering=False, debug=False,
                   num_devices=NCORES)
    ins = {
        "x": nc.dram_tensor("x", [C, HW], F32, kind="ExternalInput").ap(),
        "wpair": nc.dram_tensor("wpair", [6, 128, 18], F32, kind="ExternalInput").ap(),
        "wdrep": nc.dram_tensor("wdrep", [KK * 2, 128, 64], BF16, kind="ExternalInput").ap(),
        "tab": nc.dram_tensor("tab", [NRECP, 256], BF16, kind="Internal").ap(),
        "rdram": nc.dram_tensor("rdram", [4 * KK, 16384], BF16, kind="Internal").ap(),
    }
    outs = {"out": nc.dram_tensor("out", [C, HW], F32, kind="ExternalOutput").ap()}
    with tile.TileContext(nc, trace_sim=False) as tc:
        build_kernel(tc, outs, ins)
    nc.compile()
    _NC_CACHE["nc"] = nc
    return nc


def kernel(x, w_offset, w_deform):
    x = np.asarray(x, dtype=np.float32)
    w_offset = np.asarray(w_offset, dtype=np.float32)
    w_deform = np.asarray(w_deform, dtype=np.float32)
    nc = _build_nc()
    in_maps = [prep_core_inputs(x[b], w_offset, w_deform) for b in range(B)]
    res = bass_utils.run_bass_kernel_spmd(nc, in_maps, core_ids=list(range(NCORES)))
    out = np.stack([res.results[b]["out"].reshape(C, H, W) for b in range(B)])
    return out.astype(np.float32)

